# revision 44
# baseline (speedup 1.0000x reference)
"""CMC loss kernel for Trainium2, sharded across 8 NeuronCores.

Sharding: core i owns view d=i for the negative term (full BxB Gram of
zn[:, i, :]), and the 256-sample slice [256*i, 256*(i+1)) for the positive
term (all 28 view pairs).  Host combines per-core partial sums and does the
final (tiny) logits/logsumexp reduction.

Layout: the host pre-transposes the view slice to [F, B] fp8 (window-
major), so the kernel needs no on-chip transposes.  Per-sample norms are
column sums of squares via ones-vector matmuls on the otherwise-idle PE;
1/sqrt comes from Ln/Exp on the activation engine; a ones-matmul
partition-broadcast puts the inverse norms (x64 fp8 prescale folded in)
in PSUM, where the DVE reads them to emit the scaled fp8 operand.  The
B x B Gram runs as DoubleRow fp8 matmuls (full F=256 contraction per
instruction); the row-wise exp+sum rides the activation engine's
accumulator, writing partial sums straight into the output tile.  The
positive term works on a sample-major bf16 slice with offset-sliced pair
products on raw data (diagonal o=0 doubles as the norms), reduced on the
DVE under the exp stream and scaled by pairwise inverse-norm products.
GpSimd only triggers DMAs and memsets: its tensor ops slow concurrent
DVE work to lockstep on this part.  tile_wait_until nudges keep the
scheduler (FIFO by dependency-resolve time) from slotting the big pos
reduces ahead of the gram-critical DVE chain.
"""

import os
import sys

import numpy as np

sys.path.insert(0, "/opt/trn_rl_repo")

import concourse.bass as bass  # noqa: E402
import concourse.mybir as mybir  # noqa: E402
from concourse.bass_utils import run_bass_kernel_spmd  # noqa: E402
from concourse.library_overlay import lower_extended_insts  # noqa: E402
from concourse.tile import TileContext  # noqa: E402

import ml_dtypes  # noqa: E402


def _split_waits(nc, max_waits=1):
    """Hoist excess semaphore waits onto standalone event-sem instructions.

    Most TRN2 ISA structs only have sync slots for one wait (plus updates);
    walrus fails with "Too many sync wait commands" otherwise.  An engine
    stalls identically whether a wait rides on the instruction or on an
    InstEventSemaphore immediately before it in the same engine's stream,
    so splitting is semantics-preserving.
    """
    n = 0
    for fn in nc.m.functions:
        for bb in fn.blocks:
            out = []
            changed = False
            for inst in bb.instructions:
                si = inst.sync_info
                if si is not None and si.on_wait and len(si.on_wait) > max_waits:
                    waits = list(si.on_wait)
                    for w in waits[:-max_waits]:
                        out.append(
                            mybir.InstEventSemaphore(
                                name=f"WSPLIT-{n}",
                                engine=inst.engine,
                                ins=[],
                                outs=[],
                                sync_info=mybir.SyncInfo(
                                    on_wait=[w], on_update=[]
                                ),
                            )
                        )
                        n += 1
                    inst.sync_info = mybir.SyncInfo(
                        on_wait=waits[-max_waits:], on_update=si.on_update
                    )
                    changed = True
                out.append(inst)
            if changed:
                bb.instructions = out


B, D, F = 2048, 8, 256
NCORES = 8
BS = B // NCORES  # 256 samples per core (pos term)
P = 128
NB = B // P  # 16 row blocks of the gram
NH = F // P  # 2 feature halves
CC = 512  # matmul free-dim chunk (one PSUM bank)
NJ = BS // P  # 2 sample tiles for pos term
NPAIR = D * (D - 1) // 2  # 28 unordered view pairs
TEMP = 0.5
INV_TEMP = 1.0 / TEMP
FS = 64.0  # fp8 prescale folded into the inverse norms
NW_HOST = 4  # 512-col load windows of the view slice

# packed input layout:
ZVT_W = NH * B  # 4096: z[:, i, :].T as [P, NH, B]  (f-major)
ZS_W = NJ * D * F  # 4096: z[i*BS:(i+1)*BS] as [P, NJ, D, F] (sample-major)

f32 = mybir.dt.float32
bf16 = mybir.dt.bfloat16
fp8 = mybir.dt.float8e4
ALU = mybir.AluOpType
ACT = mybir.ActivationFunctionType

_CACHED_NC = None


def _build_nc():
    nc = bass.Bass()

    zin8 = nc.dram_tensor("zin8", [P, ZVT_W], fp8, kind="ExternalInput")
    zin16 = nc.dram_tensor("zin16", [P, ZS_W], bf16, kind="ExternalInput")
    # columns 0..NB-1: per-view gram exp row sums (incl. ~e^2 diagonal)
    # columns NB..NB+NJ-1: pos-term pair-exp sums (d<e only)
    out = nc.dram_tensor("out", [P, NB + NJ], f32, kind="ExternalOutput")

    with TileContext(nc) as tc:
        with (
            tc.tile_pool(name="singles", bufs=1) as singles,
            tc.tile_pool(name="work", bufs=3) as work,
            tc.tile_pool(name="small", bufs=4) as small,
            tc.tile_pool(name="psumB", bufs=2, space="PSUM") as psumB,
        ):
            zvt_sb = singles.tile([P, ZVT_W], fp8)
            zs_sb = singles.tile([P, ZS_W], bf16)
            # window-major fp8 zvT: one contiguous DMA per 512-col window
            # spread over three trigger queues; bf16 pos slice loads last
            NW = B // CC  # 4 windows
            WW = NH * CC  # 1024 elements per window (both halves)
            qs = [nc.sync, nc.scalar, nc.gpsimd]
            for w in range(NW):
                qs[w % 3].dma_start(
                    out=zvt_sb[:, w * WW : (w + 1) * WW],
                    in_=zin8[:, w * WW : (w + 1) * WW],
                )
            nc.gpsimd.dma_start(out=zs_sb[:, :], in_=zin16[:, :])

            # zvw[p, w, h, c]: window-major view of the view slice
            zvw = zvt_sb.rearrange("p (w h c) -> p w h c", w=NW, h=NH)
            zs = zs_sb.rearrange("p (j d f) -> p j d f", j=NJ, d=D)

            ones_c = singles.tile([P, 1], bf16)  # colsum weights [128, 1]
            nc.gpsimd.memset(ones_c[:, :], 1.0)
            ones_r = singles.tile([1, P], bf16)  # bcast weights [1, 128]
            nc.gpsimd.memset(ones_r[:, :], 1.0)
            # dummy ACT to hoist the activation-table load off the
            # critical path (runs during the input DMA)
            actwarm = small.tile([P, 1], f32)
            nc.scalar.activation(actwarm, ones_c, ACT.Ln)

            # --- view norms + fp8 scale, pipelined by 512-col window ---
            # per window: square (DVE) -> colsum ones-matmul (PE) ->
            # Ln/Exp (ACT) -> ones-matmul broadcast back into the same
            # PSUM tile (write-after-read) -> DVE scales to fp8 reading
            # the broadcast PSUM directly
            sq = work.tile([P, NW, NH, CC], bf16, tag="sq")
            n2f = psumB.tile([P, B], f32, tag="ps", name="n2f")
            n2p = n2f[0:1, :]
            lnb = small.tile([1, B], f32)
            invrow = small.tile([1, B], bf16)
            zts = singles.tile([P, NH, B], fp8)

            for w2 in range(2):
                for h in range(NH):
                    nc.vector.tensor_mul(
                        sq[:, 2 * w2 : 2 * w2 + 2, h, :],
                        zvw[:, 2 * w2 : 2 * w2 + 2, h, :],
                        zvw[:, 2 * w2 : 2 * w2 + 2, h, :],
                    )
            # two half-width broadcast PSUM tiles: tile-level deps then
            # release the first zts pair after only two bcast matmuls
            invhalf = [
                psumB.tile([P, B // 2], f32, tag="ps", name=f"invbp{x}")
                for x in range(2)
            ]
            for w in range(2):
                ws = slice(w * 1024, (w + 1) * 1024)
                for c2 in range(2):
                    cs = slice(w * 1024 + c2 * CC, w * 1024 + (c2 + 1) * CC)
                    for h in range(NH):
                        nc.tensor.matmul(
                            n2p[:, cs],
                            ones_c,
                            sq[:, 2 * w + c2, h, :],
                            start=(h == 0),
                            stop=(h == NH - 1),
                        )
                # invrow = FS/sqrt(n2) = exp(-0.5 ln(n2/FS^2)), [1, 1024]
                nc.scalar.activation(
                    lnb[:, ws], n2p[:, ws], ACT.Ln, scale=1.0 / (FS * FS)
                )
                nc.scalar.activation(
                    invrow[:, ws], lnb[:, ws], ACT.Exp, scale=-0.5
                )
                # partition-broadcast via ones-matmuls into PSUM; the zts
                # scale reads the PSUM window directly
                for c2 in range(2):
                    cs = slice(w * 1024 + c2 * CC, w * 1024 + (c2 + 1) * CC)
                    nc.tensor.matmul(
                        invhalf[w][:, c2 * CC : (c2 + 1) * CC],
                        ones_r, invrow[:, cs],
                        start=True, stop=True,
                    )
            for w2 in range(2):
                ws = slice(w2 * 1024, (w2 + 1) * 1024)
                ihv = invhalf[w2].rearrange("p (x c) -> p x c", x=2)
                for h in range(NH):
                    nc.vector.tensor_mul(
                        zts[:, h, ws].rearrange("p (x c) -> p x c", x=2),
                        zvw[:, 2 * w2 : 2 * w2 + 2, h, :],
                        ihv,
                    )

            # --- pos pair products on RAW data, incl. o=0 diagonal ---
            # segment s of 36: o=0 -> s=0..7 are zs[d]^2 (norms), then
            # offset-o pairs (d, d+o) at s = 8 + ofs(o) + d
            # GpSimd (otherwise idle) computes o=2..7 early; the DVE part
            # is emitted after the gram loop so the scheduler keeps the
            # gram-critical DVE chain first
            NSEG = D + NPAIR  # 36
            prod = singles.tile([P, NJ, NSEG, F], bf16)
            # all products on DVE (concurrent GpSimd tensor ops slow DVE
            # to lockstep on this part — keep GpSimd idle).  The wait
            # delays when the products (and so the big reduces depending
            # on them) become schedulable: the tile scheduler dispatches
            # FIFO by dependency-resolve time, and without the delay the
            # products and 8-11us reduces jump ahead of the gram-critical
            # zts ops on the DVE queue.  Per-(j,o) ops keep every piece a
            # contiguous 2D access (2x DVE rate) and small enough to not
            # block the queue for long.
            with tc.tile_wait_until(0.013):
                for j in range(NJ):
                    nc.vector.tensor_mul(
                        prod[:, j, 0:D, :], zs[:, j, :, :], zs[:, j, :, :]
                    )
                    for o in range(1, D):
                        wd = D - o
                        ofs = D + (o - 1) * (2 * D - o) // 2
                        nc.vector.tensor_mul(
                            prod[:, j, ofs : ofs + wd, :],
                            zs[:, j, 0:wd, :],
                            zs[:, j, o:D, :],
                        )
            rawdot = small.tile([P, NJ, NSEG], bf16)

            # accum_out targets write straight into the output tile
            outsb = singles.tile([P, NB + NJ], f32)

            lns = small.tile([P, NJ, D], f32)
            invs = small.tile([P, NJ, D], f32)
            invprod = small.tile([P, NJ, NPAIR], f32)
            sdots = small.tile([P, NJ, NPAIR], f32)
            pjunk = small.tile([P, NJ, NPAIR], bf16)

            # --- gram: fp8 DoubleRow matmuls, full F contraction each ---
            for rb in range(NB):
                ps = psumB.tile([P, B], f32, tag="ps")
                for c in range(B // CC):
                    nc.tensor.matmul(
                        ps[:, c * CC : (c + 1) * CC],
                        zts[:, :, rb * P : (rb + 1) * P],
                        zts[:, :, c * CC : (c + 1) * CC],
                        start=True,
                        stop=True,
                        perf_mode=mybir.MatmulPerfMode.DoubleRow,
                    )
                ejunk = work.tile([P, B], fp8, tag="ejunk")
                nc.scalar.activation(
                    ejunk, ps, ACT.Exp, scale=INV_TEMP / (FS * FS),
                    accum_out=outsb[:, rb : rb + 1],
                )
                if rb == NB - 2:
                    # pos exp + accumulate, slotted before the last gram
                    # exp so the tail is just one exp + the output DMA
                    for j in range(NJ):
                        nc.scalar.activation(
                            pjunk[:, j, :], sdots[:, j, :], ACT.Exp,
                            scale=INV_TEMP,
                            accum_out=outsb[:, NB + j : NB + j + 1],
                        )
                if rb == 10:
                    # pos-term reduces + tiny norm ACTs; the wait_until
                    # keeps the scheduler from slotting these big DVE ops
                    # ahead of the gram-critical zts scale chain
                    with tc.tile_wait_until(0.014):
                        with nc.allow_low_precision(
                            reason="pair dots |.|<40 bf16; final tol 2e-2"
                        ):
                            # per-j 3D reduces (last dim contiguous)
                            for j in range(NJ):
                                nc.vector.tensor_reduce(
                                    out=rawdot[:, j, 0 : D + 7],
                                    in_=prod[:, j, 0 : D + 7, :],
                                    axis=mybir.AxisListType.X, op=ALU.add,
                                )
                                nc.vector.tensor_reduce(
                                    out=rawdot[:, j, D + 7 :],
                                    in_=prod[:, j, D + 7 :, :],
                                    axis=mybir.AxisListType.X, op=ALU.add,
                                )
                        nc.scalar.activation(
                            lns, rawdot[:, :, 0:D], ACT.Ln
                        )
                        nc.scalar.activation(
                            invs, lns, ACT.Exp, scale=-0.5
                        )
                        ofs = 0
                        for o in range(1, D):
                            wd = D - o
                            nc.vector.tensor_mul(
                                invprod[:, :, ofs : ofs + wd],
                                invs[:, :, 0:wd],
                                invs[:, :, o:D],
                            )
                            ofs += wd
                        nc.vector.tensor_mul(
                            sdots, rawdot[:, :, D:], invprod
                        )

            nc.sync.dma_start(out=out[:, :], in_=outsb)

    _insert_library_loads(nc)
    if os.environ.get("KERNEL_NO_SPLIT") != "1":  # CoreSim can't run the
        _split_waits(nc)  # post-hoc event-sem instructions; HW needs them
    lower_extended_insts(nc)
    return nc


def _insert_library_loads(nc):
    """GpSimd library loads for partition_all_reduce (attn library).

    Same pass Bacc.compile runs; raw Bass skips it, but the Pool
    all-reduce is an extended inst that needs its ucode library resident.
    """
    import bass_rust as _bass_rust
    from concourse.library_config import all_libraries, standard

    inst_type_to_lib_mask = {}
    for lib in all_libraries:
        for inst_type in lib.instructions:
            inst_type_to_lib_mask[inst_type] = inst_type_to_lib_mask.get(
                inst_type, 0
            ) | (1 << lib.index)
    _bass_rust.insert_library_loads(
        nc, inst_type_to_lib_mask, len(all_libraries), standard.index
    )


def _get_nc():
    global _CACHED_NC
    if _CACHED_NC is None:
        _CACHED_NC = _build_nc()
    return _CACHED_NC


def _pack_core_input(z, i):
    # view slice, window-major fp8: zvt[p, w, h, c] = z[w*512+c, i, 128h+p]
    zvt = (
        z[:, i, :]
        .T.reshape(NH, P, NW_HOST, CC)
        .transpose(1, 2, 0, 3)
        .reshape(P, ZVT_W)
    )
    # pos slice, sample-major bf16: zs[p,j,d,f] = z[i*BS + j*128 + p, d, f]
    zsl = (
        z[i * BS : (i + 1) * BS]
        .reshape(NJ, P, D, F)
        .transpose(1, 0, 2, 3)
        .reshape(P, ZS_W)
    )
    return {
        "zin8": np.ascontiguousarray(zvt.astype(ml_dtypes.float8_e4m3)),
        "zin16": np.ascontiguousarray(zsl.astype(ml_dtypes.bfloat16)),
    }


def _run(z, trace=False):
    z = np.ascontiguousarray(np.asarray(z, dtype=np.float32))
    assert z.shape == (B, D, F), z.shape
    in_maps = [_pack_core_input(z, i) for i in range(NCORES)]
    nc = _get_nc()
    res = run_bass_kernel_spmd(
        nc, in_maps, core_ids=list(range(NCORES)), trace=trace
    )
    return res


def _finish(results):
    neg_raw = np.zeros(B, np.float64)
    pos_half = np.zeros(B, np.float64)
    for i, r in enumerate(results):
        o = np.asarray(r["out"], np.float64)  # [P, NB + NJ]
        rowsums = o[:, :NB]  # [P, NB] ; sample = t*128 + p
        possums = o[:, NB:]  # [P, NJ] ; sample = i*BS + j*128 + p
        neg_raw += rowsums.T.reshape(B)
        pos_half[i * BS : (i + 1) * BS] = possums.T.reshape(BS)

    e2 = np.exp(INV_TEMP)  # exp(1/T * 1.0) diagonal term
    neg = (neg_raw - D * e2) / (B - 1)
    pos = 2.0 * pos_half
    logits = pos / (pos + neg)
    m = logits.max()
    lse = np.log(np.sum(np.exp(logits - m))) + m
    loss = lse - logits.mean()
    return np.float32(loss)


def kernel(**inputs) -> np.ndarray:
    res = _run(inputs["z"], trace=False)
    return _finish(res.results)


# revision 48
# speedup vs baseline: 1.0069x; 1.0069x over previous
"""CMC loss kernel for Trainium2, sharded across 8 NeuronCores.

Sharding: core i owns view d=i for the negative term (full BxB Gram of
zn[:, i, :]), and the 256-sample slice [256*i, 256*(i+1)) for the positive
term (all 28 view pairs).  Host combines per-core partial sums and does the
final (tiny) logits/logsumexp reduction.

Layout: the host pre-transposes the view slice to [F, B] fp8 (window-
major), so the kernel needs no on-chip transposes.  Per-sample norms are
column sums of squares via ones-vector matmuls on the otherwise-idle PE;
1/sqrt comes from Ln/Exp on the activation engine; a ones-matmul
partition-broadcast puts the inverse norms (x64 fp8 prescale folded in)
in PSUM, where the DVE reads them to emit the scaled fp8 operand.  The
B x B Gram runs as DoubleRow fp8 matmuls (full F=256 contraction per
instruction); the row-wise exp+sum rides the activation engine's
accumulator, writing partial sums straight into the output tile.  The
positive term works on a sample-major bf16 slice with offset-sliced pair
products on raw data (diagonal o=0 doubles as the norms), reduced on the
DVE under the exp stream and scaled by pairwise inverse-norm products.
GpSimd only triggers DMAs and memsets: its tensor ops slow concurrent
DVE work to lockstep on this part.  tile_wait_until nudges keep the
scheduler (FIFO by dependency-resolve time) from slotting the big pos
reduces ahead of the gram-critical DVE chain.
"""

import os
import sys

import numpy as np

sys.path.insert(0, "/opt/trn_rl_repo")

import concourse.bass as bass  # noqa: E402
import concourse.mybir as mybir  # noqa: E402
from concourse.bass_utils import run_bass_kernel_spmd  # noqa: E402
from concourse.library_overlay import lower_extended_insts  # noqa: E402
from concourse.tile import TileContext  # noqa: E402

import ml_dtypes  # noqa: E402


def _split_waits(nc, max_waits=1):
    """Hoist excess semaphore waits onto standalone event-sem instructions.

    Most TRN2 ISA structs only have sync slots for one wait (plus updates);
    walrus fails with "Too many sync wait commands" otherwise.  An engine
    stalls identically whether a wait rides on the instruction or on an
    InstEventSemaphore immediately before it in the same engine's stream,
    so splitting is semantics-preserving.
    """
    n = 0
    for fn in nc.m.functions:
        for bb in fn.blocks:
            out = []
            changed = False
            for inst in bb.instructions:
                si = inst.sync_info
                if si is not None and si.on_wait and len(si.on_wait) > max_waits:
                    waits = list(si.on_wait)
                    for w in waits[:-max_waits]:
                        out.append(
                            mybir.InstEventSemaphore(
                                name=f"WSPLIT-{n}",
                                engine=inst.engine,
                                ins=[],
                                outs=[],
                                sync_info=mybir.SyncInfo(
                                    on_wait=[w], on_update=[]
                                ),
                            )
                        )
                        n += 1
                    inst.sync_info = mybir.SyncInfo(
                        on_wait=waits[-max_waits:], on_update=si.on_update
                    )
                    changed = True
                out.append(inst)
            if changed:
                bb.instructions = out


B, D, F = 2048, 8, 256
NCORES = 8
BS = B // NCORES  # 256 samples per core (pos term)
P = 128
NB = B // P  # 16 row blocks of the gram
NH = F // P  # 2 feature halves
CC = 512  # matmul free-dim chunk (one PSUM bank)
NJ = BS // P  # 2 sample tiles for pos term
NPAIR = D * (D - 1) // 2  # 28 unordered view pairs
TEMP = 0.5
INV_TEMP = 1.0 / TEMP
FS = 64.0  # fp8 prescale folded into the inverse norms
NW_HOST = 4  # 512-col load windows of the view slice

# packed input layout:
ZVT_W = NH * B  # 4096: z[:, i, :].T as [P, NH, B]  (f-major)
ZS_W = NJ * D * F  # 4096: z[i*BS:(i+1)*BS] as [P, NJ, D, F] (sample-major)

f32 = mybir.dt.float32
bf16 = mybir.dt.bfloat16
fp8 = mybir.dt.float8e4
ALU = mybir.AluOpType
ACT = mybir.ActivationFunctionType

_CACHED_NC = None


def _build_nc():
    nc = bass.Bass()

    zin8 = nc.dram_tensor("zin8", [P, ZVT_W], fp8, kind="ExternalInput")
    zin16 = nc.dram_tensor("zin16", [P, ZS_W], bf16, kind="ExternalInput")
    # columns 0..NB-1: per-view gram exp row sums (incl. ~e^2 diagonal)
    # columns NB..NB+NJ-1: pos-term pair-exp sums (d<e only)
    out = nc.dram_tensor("out", [P, NB + NJ], f32, kind="ExternalOutput")

    with TileContext(nc) as tc:
        with (
            tc.tile_pool(name="singles", bufs=1) as singles,
            tc.tile_pool(name="work", bufs=3) as work,
            tc.tile_pool(name="small", bufs=4) as small,
            tc.tile_pool(name="psumB", bufs=2, space="PSUM") as psumB,
        ):
            zvt_sb = singles.tile([P, ZVT_W], fp8)
            zs_sb = singles.tile([P, ZS_W], bf16)
            # window-major fp8 zvT: one contiguous DMA per 512-col window
            # spread over three trigger queues; bf16 pos slice loads last
            NW = B // CC  # 4 windows
            WW = NH * CC  # 1024 elements per window (both halves)
            qs = [nc.sync, nc.scalar, nc.gpsimd]
            for w in range(NW):
                qs[w % 3].dma_start(
                    out=zvt_sb[:, w * WW : (w + 1) * WW],
                    in_=zin8[:, w * WW : (w + 1) * WW],
                )
            nc.gpsimd.dma_start(out=zs_sb[:, :], in_=zin16[:, :])

            # zvw[p, w, h, c]: window-major view of the view slice
            zvw = zvt_sb.rearrange("p (w h c) -> p w h c", w=NW, h=NH)
            zs = zs_sb.rearrange("p (j d f) -> p j d f", j=NJ, d=D)

            ones_c = singles.tile([P, 1], bf16)  # colsum weights [128, 1]
            nc.gpsimd.memset(ones_c[:, :], 1.0)
            ones_r = singles.tile([1, P], bf16)  # bcast weights [1, 128]
            nc.gpsimd.memset(ones_r[:, :], 1.0)
            # dummy ACT to hoist the activation-table load off the
            # critical path (runs during the input DMA)
            actwarm = small.tile([P, 1], f32)
            nc.scalar.activation(actwarm, ones_c, ACT.Ln)

            # --- view norms + fp8 scale, pipelined by 512-col window ---
            # per window: square (DVE) -> colsum ones-matmul (PE) ->
            # Ln/Exp (ACT) -> ones-matmul broadcast back into the same
            # PSUM tile (write-after-read) -> DVE scales to fp8 reading
            # the broadcast PSUM directly
            sq = work.tile([P, NW, NH, CC], bf16, tag="sq")
            n2f = psumB.tile([P, B], f32, tag="ps", name="n2f")
            n2p = n2f[0:1, :]
            lnb = small.tile([1, B], f32)
            invrow = small.tile([1, B], bf16)
            zts = singles.tile([P, NH, B], fp8)

            for w2 in range(2):
                for h in range(NH):
                    nc.vector.tensor_mul(
                        sq[:, 2 * w2 : 2 * w2 + 2, h, :],
                        zvw[:, 2 * w2 : 2 * w2 + 2, h, :],
                        zvw[:, 2 * w2 : 2 * w2 + 2, h, :],
                    )
            # two half-width broadcast PSUM tiles: tile-level deps then
            # release the first zts pair after only two bcast matmuls
            invhalf = [
                psumB.tile([P, B // 2], f32, tag="ps", name=f"invbp{x}")
                for x in range(2)
            ]
            for w in range(2):
                ws = slice(w * 1024, (w + 1) * 1024)
                for c2 in range(2):
                    cs = slice(w * 1024 + c2 * CC, w * 1024 + (c2 + 1) * CC)
                    for h in range(NH):
                        nc.tensor.matmul(
                            n2p[:, cs],
                            ones_c,
                            sq[:, 2 * w + c2, h, :],
                            start=(h == 0),
                            stop=(h == NH - 1),
                        )
                # invrow = FS/sqrt(n2) = exp(-0.5 ln(n2/FS^2)), [1, 1024]
                nc.scalar.activation(
                    lnb[:, ws], n2p[:, ws], ACT.Ln, scale=1.0 / (FS * FS)
                )
                nc.scalar.activation(
                    invrow[:, ws], lnb[:, ws], ACT.Exp, scale=-0.5
                )
                # partition-broadcast via ones-matmuls into PSUM; the zts
                # scale reads the PSUM window directly
                for c2 in range(2):
                    cs = slice(w * 1024 + c2 * CC, w * 1024 + (c2 + 1) * CC)
                    nc.tensor.matmul(
                        invhalf[w][:, c2 * CC : (c2 + 1) * CC],
                        ones_r, invrow[:, cs],
                        start=True, stop=True,
                    )
            for w2 in range(2):
                ws = slice(w2 * 1024, (w2 + 1) * 1024)
                ihv = invhalf[w2].rearrange("p (x c) -> p x c", x=2)
                for h in range(NH):
                    nc.vector.tensor_mul(
                        zts[:, h, ws].rearrange("p (x c) -> p x c", x=2),
                        zvw[:, 2 * w2 : 2 * w2 + 2, h, :],
                        ihv,
                    )

            # --- pos pair products on RAW data, incl. o=0 diagonal ---
            # segment s of 36: o=0 -> s=0..7 are zs[d]^2 (norms), then
            # offset-o pairs (d, d+o) at s = 8 + ofs(o) + d
            # GpSimd (otherwise idle) computes o=2..7 early; the DVE part
            # is emitted after the gram loop so the scheduler keeps the
            # gram-critical DVE chain first
            NSEG = D + NPAIR  # 36
            prod = singles.tile([P, NJ, NSEG, F], bf16)
            # all products on DVE (concurrent GpSimd tensor ops slow DVE
            # to lockstep on this part — keep GpSimd idle).  The wait
            # delays when the products (and so the big reduces depending
            # on them) become schedulable: the tile scheduler dispatches
            # FIFO by dependency-resolve time, and without the delay the
            # products and 8-11us reduces jump ahead of the gram-critical
            # zts ops on the DVE queue.  Per-(j,o) ops keep every piece a
            # contiguous 2D access (2x DVE rate) and small enough to not
            # block the queue for long.
            with tc.tile_wait_until(0.013):
                for j in range(NJ):
                    nc.vector.tensor_mul(
                        prod[:, j, 0:D, :], zs[:, j, :, :], zs[:, j, :, :]
                    )
                    for o in range(1, D):
                        wd = D - o
                        ofs = D + (o - 1) * (2 * D - o) // 2
                        nc.vector.tensor_mul(
                            prod[:, j, ofs : ofs + wd, :],
                            zs[:, j, 0:wd, :],
                            zs[:, j, o:D, :],
                        )
            rawdot = small.tile([P, NJ, NSEG], bf16)

            # accum_out targets write straight into the output tile
            outsb = singles.tile([P, NB + NJ], f32)

            lns = small.tile([P, NJ, D], f32)
            invs = small.tile([P, NJ, D], f32)
            invprod = small.tile([P, NJ, NPAIR], f32)
            sdots = small.tile([P, NJ, NPAIR], f32)
            pjunk = small.tile([P, NJ, NPAIR], bf16)

            # --- gram: fp8 DoubleRow matmuls, full F contraction each ---
            for rb in range(NB):
                ps = psumB.tile([P, B], f32, tag="ps")
                for c in range(B // CC):
                    nc.tensor.matmul(
                        ps[:, c * CC : (c + 1) * CC],
                        zts[:, :, rb * P : (rb + 1) * P],
                        zts[:, :, c * CC : (c + 1) * CC],
                        start=True,
                        stop=True,
                        perf_mode=mybir.MatmulPerfMode.DoubleRow,
                    )
                ejunk = work.tile([P, B], fp8, tag="ejunk")
                nc.scalar.activation(
                    ejunk, ps, ACT.Exp, scale=INV_TEMP / (FS * FS),
                    accum_out=outsb[:, rb : rb + 1],
                )
                if rb == NB - 2:
                    # pos exp + accumulate, slotted before the last gram
                    # exp so the tail is just one exp + the output DMA
                    for j in range(NJ):
                        nc.scalar.activation(
                            pjunk[:, j, :], sdots[:, j, :], ACT.Exp,
                            scale=INV_TEMP,
                            accum_out=outsb[:, NB + j : NB + j + 1],
                        )
                if rb == 10:
                    # pos-term reduces + tiny norm ACTs; the wait_until
                    # keeps the scheduler from slotting these big DVE ops
                    # ahead of the gram-critical zts scale chain
                    with tc.tile_wait_until(0.014):
                        with nc.allow_low_precision(
                            reason="pair dots |.|<40 bf16; final tol 2e-2"
                        ):
                            # per-j 3D reduces (last dim contiguous)
                            for j in range(NJ):
                                nc.vector.tensor_reduce(
                                    out=rawdot[:, j, 0 : D + 7],
                                    in_=prod[:, j, 0 : D + 7, :],
                                    axis=mybir.AxisListType.X, op=ALU.add,
                                )
                                nc.vector.tensor_reduce(
                                    out=rawdot[:, j, D + 7 :],
                                    in_=prod[:, j, D + 7 :, :],
                                    axis=mybir.AxisListType.X, op=ALU.add,
                                )
                        nc.scalar.activation(
                            lns, rawdot[:, :, 0:D], ACT.Ln
                        )
                        nc.scalar.activation(
                            invs, lns, ACT.Exp, scale=-0.5
                        )
                        ofs = 0
                        for o in range(1, D):
                            wd = D - o
                            nc.vector.tensor_mul(
                                invprod[:, :, ofs : ofs + wd],
                                invs[:, :, 0:wd],
                                invs[:, :, o:D],
                            )
                            ofs += wd
                        nc.vector.tensor_mul(
                            sdots, rawdot[:, :, D:], invprod
                        )

            nc.sync.dma_start(out=out[:, :], in_=outsb)

    _insert_library_loads(nc)
    if os.environ.get("KERNEL_NO_SPLIT") != "1":  # CoreSim can't run the
        _split_waits(nc)  # post-hoc event-sem instructions; HW needs them
    lower_extended_insts(nc)
    return nc


def _insert_library_loads(nc):
    """GpSimd library loads for partition_all_reduce (attn library).

    Same pass Bacc.compile runs; raw Bass skips it, but the Pool
    all-reduce is an extended inst that needs its ucode library resident.
    """
    import bass_rust as _bass_rust
    from concourse.library_config import all_libraries, standard

    inst_type_to_lib_mask = {}
    for lib in all_libraries:
        for inst_type in lib.instructions:
            inst_type_to_lib_mask[inst_type] = inst_type_to_lib_mask.get(
                inst_type, 0
            ) | (1 << lib.index)
    _bass_rust.insert_library_loads(
        nc, inst_type_to_lib_mask, len(all_libraries), standard.index
    )


def _get_nc():
    global _CACHED_NC
    if _CACHED_NC is None:
        _CACHED_NC = _build_nc()
    return _CACHED_NC


def _pack_core_input(z, i):
    # view slice, window-major fp8: zvt[p, w, h, c] = z[w*512+c, i, 128h+p]
    zvt = (
        z[:, i, :]
        .T.reshape(NH, P, NW_HOST, CC)
        .transpose(1, 2, 0, 3)
        .reshape(P, ZVT_W)
    )
    # pos slice, sample-major bf16: zs[p,j,d,f] = z[i*BS + j*128 + p, d, f]
    zsl = (
        z[i * BS : (i + 1) * BS]
        .reshape(NJ, P, D, F)
        .transpose(1, 0, 2, 3)
        .reshape(P, ZS_W)
    )
    return {
        "zin8": np.ascontiguousarray(zvt.astype(ml_dtypes.float8_e4m3)),
        "zin16": np.ascontiguousarray(zsl.astype(ml_dtypes.bfloat16)),
    }


def _run(z, trace=False):
    z = np.ascontiguousarray(np.asarray(z, dtype=np.float32))
    assert z.shape == (B, D, F), z.shape
    in_maps = [_pack_core_input(z, i) for i in range(NCORES)]
    nc = _get_nc()
    res = run_bass_kernel_spmd(
        nc, in_maps, core_ids=list(range(NCORES)), trace=trace
    )
    return res


def _finish(results):
    neg_raw = np.zeros(B, np.float64)
    pos_half = np.zeros(B, np.float64)
    for i, r in enumerate(results):
        o = np.asarray(r["out"], np.float64)  # [P, NB + NJ]
        rowsums = o[:, :NB]  # [P, NB] ; sample = t*128 + p
        possums = o[:, NB:]  # [P, NJ] ; sample = i*BS + j*128 + p
        neg_raw += rowsums.T.reshape(B)
        pos_half[i * BS : (i + 1) * BS] = possums.T.reshape(BS)

    e2 = np.exp(INV_TEMP)  # exp(1/T * 1.0) diagonal term
    neg = (neg_raw - D * e2) / (B - 1)
    pos = 2.0 * pos_half
    logits = pos / (pos + neg)
    m = logits.max()
    lse = np.log(np.sum(np.exp(logits - m))) + m
    loss = lse - logits.mean()
    return np.float32(loss)


def kernel(**inputs) -> np.ndarray:
    res = _run(inputs["z"], trace=False)
    return _finish(res.results)
